# revision 2
# baseline (speedup 1.0000x reference)
"""Multi-head attention (B=2, S=2048, D=1024, H=16) on 8 trn2 NeuronCores.

Sharding: core c handles batch b=c//4 and query rows [512*(c%4), +512).
K/V projection is sharded across the 4 cores of each batch group: core c
projects key/value rows of its own chunk L=c%4 only, then the projected
K.T / V chunks are exchanged with a 4-core AllGather. The AG flies while
the core runs attention on its locally-projected chunk, so its latency is
hidden. Attention accumulates unnormalized head outputs (plus a softmax
denominator row) into an SBUF accumulator across the 4 key chunks.

All device layouts are feature-major so no on-chip transposes are needed:
  - inputs passed as query.T/key.T/value.T [D, rows], weights as W.T [in, out]
  - projections produce Q.T/K.T [out_feat, rows] and V [rows, out_feat]
  - scores computed transposed [k, q]; softmax across k (partitions):
      exp on ACT with the mask folded into the per-partition exp bias,
      denominator via a ones-row appended to V in the attn@V matmul,
      normalization via a K=1 broadcast matmul + one DVE multiply
  - 1/sqrt(dk) folded into wq host-side; bv/bo folded into bo' = bo + wo@bv
"""

import sys

for _p in ("/opt/trn_rl_repo", "/root/.axon_site/_ro/trn_rl_repo"):
    if _p not in sys.path:
        sys.path.insert(0, _p)

import numpy as np
import ml_dtypes

B, S, D, H, DK = 2, 2048, 1024, 16, 64
NCORES = 8
MQ = 512          # query rows per core
P = 128           # partitions
NQT = MQ // P     # 4 query row-tiles
NIT = D // P      # 8 input-feature tiles
NOT_ = D // P     # 8 output-feature tiles
NKT = S // P      # 16 key tiles
NKC = S // 512    # 4 key chunks of 512
VW = DK + 1       # 65: head dim + ones row
KHALF = D * 512   # flat elements of one K.T chunk
KHALFV = 512 * H * VW  # flat elements of one V' chunk (ones column included)

BF16 = ml_dtypes.bfloat16

_CACHE = {}


def _build(loop_n=None, phases=("q", "kv", "attn", "out"), sim_noag=False):
    from concourse import bacc
    import concourse.mybir as mybir
    import concourse.tile as tile
    import concourse.bass as bass

    phases = frozenset(phases)
    nc = bacc.Bacc("TRN2", target_bir_lowering=False, debug=False)
    dt = mybir.dt

    qT = nc.dram_tensor("qT", [D, MQ], dt.bfloat16, kind="ExternalInput")
    kT = nc.dram_tensor("kT", [D, 512], dt.bfloat16, kind="ExternalInput")
    vT = nc.dram_tensor("vT", [D, 512], dt.bfloat16, kind="ExternalInput")
    wq = nc.dram_tensor("wq", [D, D], dt.bfloat16, kind="ExternalInput")
    wk = nc.dram_tensor("wk", [D, D], dt.bfloat16, kind="ExternalInput")
    wv = nc.dram_tensor("wv", [D, D], dt.bfloat16, kind="ExternalInput")
    wo = nc.dram_tensor("wo", [D, D], dt.bfloat16, kind="ExternalInput")
    bq = nc.dram_tensor("bq", [P, NOT_], dt.float32, kind="ExternalInput")
    bk = nc.dram_tensor("bk", [P, NOT_], dt.float32, kind="ExternalInput")
    maskb = nc.dram_tensor("maskb", [P, NKT], dt.float32, kind="ExternalInput")
    bob = nc.dram_tensor("bob", [1, D], dt.float32, kind="ExternalInput")
    onesr = nc.dram_tensor("onesr", [1, DK], dt.float32r, kind="ExternalInput")
    # local chunk id L = c%4, one-hot over psum... passed as data: simplest is
    # 4 separate branches compiled? No: L enters only via which ag_out slot is
    # "local"; we instead always compute the local chunk into slot L via the
    # AllGather (rank order == chunk order), and read every chunk back from
    # ag_out. The local slice additionally starts early from the SBUF copy.
    out = nc.dram_tensor("out", [MQ, D], dt.float32, kind="ExternalOutput")

    agk_in = nc.dram_tensor("agk_in", [KHALF], dt.bfloat16)
    agk_out = nc.dram_tensor("agk_out", [NKC * KHALF], dt.bfloat16)
    agv_in = nc.dram_tensor("agv_in", [KHALFV], dt.bfloat16)
    agv_out = nc.dram_tensor("agv_out", [NKC * KHALFV], dt.bfloat16)

    with tile.TileContext(nc) as tc:
        with (
            tc.tile_pool(name="w", bufs=3) as wpool,
            tc.tile_pool(name="stat", bufs=1) as stat,
            tc.tile_pool(name="inT", bufs=2) as inpool,
            tc.tile_pool(name="qin", bufs=1) as qin,
            tc.tile_pool(name="big", bufs=1) as big,
            tc.tile_pool(name="kc", bufs=2) as kcpool,
            tc.tile_pool(name="vc", bufs=2) as vcpool,
            tc.tile_pool(name="pT", bufs=9) as ppool,
            tc.tile_pool(name="sm", bufs=4) as sm,
            tc.tile_pool(name="outp", bufs=2) as outp,
            tc.tile_pool(name="psA", bufs=4, space="PSUM") as psA,
            tc.tile_pool(name="psB", bufs=2, space="PSUM") as psB,
        ):
            _body_ctx = (
                tc.For_i(
                    0,
                    loop_n,
                    1,
                    hint_engines=(
                        mybir.EngineType.PE,
                        mybir.EngineType.Activation,
                        mybir.EngineType.DVE,
                        mybir.EngineType.SP,
                    ),
                )
                if loop_n is not None
                else None
            )

            # ---- persistent tiles ----
            QT_sb = big.tile([P, NOT_, MQ], dt.bfloat16, tag="QT")
            ctx_sb = big.tile([P, NOT_, MQ], dt.bfloat16, tag="ctx")
            # attn@V accumulators: [0:DK] = unnormalized out_h.T, row DK = denom
            av_acc = big.tile([P, H, MQ], dt.float32, tag="avacc")
            bq_sb = stat.tile([P, NOT_], dt.float32, tag="bq")
            bk_sb = stat.tile([P, NOT_], dt.float32, tag="bk")
            mb_sb = stat.tile([P, NKT], dt.float32, tag="mb")
            bob_sb = stat.tile([P, D], dt.float32, tag="bob")
            ones_sb = stat.tile([1, DK], dt.float32r, tag="ones")

            nc.sync.dma_start(out=bq_sb, in_=bq[:, :])
            nc.sync.dma_start(out=bk_sb, in_=bk[:, :])
            nc.sync.dma_start(out=mb_sb, in_=maskb[:, :])
            bob_bcast = bass.AP(
                tensor=bob.ap().tensor, offset=0, ap=[[0, P], [1, D]]
            )
            nc.sync.dma_start(out=bob_sb, in_=bob_bcast)
            nc.sync.dma_start(out=ones_sb, in_=onesr[:, :])

            def load_w(name, dram):
                t = wpool.tile([P, NIT, D], dt.bfloat16, tag="w", name=name)
                nc.sync.dma_start(
                    out=t, in_=dram.ap().rearrange("(t p) o -> p t o", p=P)
                )
                return t

            def proj_group(ps, w_sb, x_sb, m_slice, n_slice, swap=False):
                for it in range(NIT):
                    lhsT = (
                        x_sb[:, it, m_slice] if swap else w_sb[:, it, m_slice]
                    )
                    rhs = x_sb[:, it, n_slice] if not swap else w_sb[:, it, n_slice]
                    nc.tensor.matmul(
                        ps,
                        lhsT=lhsT,
                        rhs=rhs,
                        start=(it == 0),
                        stop=(it == NIT - 1),
                    )

            def emit_body(do_ag=True):
              if "kv" in phases:
                # ---- local K chunk projection + AllGather launch ----
                wk_sb = load_w("wk_sb", wk)
                kTl = inpool.tile([P, NIT, 512], dt.bfloat16, tag="inT")
                nc.sync.dma_start(
                    out=kTl, in_=kT.ap().rearrange("(t p) k -> p t k", p=P)
                )
                KTl = kcpool.tile([P, NOT_, 512], dt.bfloat16, tag="KTc")
                for ot in range(NOT_):
                    ps = psA.tile([P, 512], dt.float32, tag="ps1")
                    proj_group(
                        ps, wk_sb, kTl, slice(ot * P, (ot + 1) * P), slice(None)
                    )
                    nc.vector.tensor_scalar_add(
                        out=KTl[:, ot, :], in0=ps, scalar1=bk_sb[:, ot : ot + 1]
                    )
                nc.sync.dma_start(
                    out=agk_in.ap().rearrange("(t p k) -> p t k", p=P, k=512),
                    in_=KTl,
                )
                if do_ag:
                    nc.gpsimd.collective_compute(
                        "AllGather",
                        mybir.AluOpType.bypass,
                        ins=[agk_in[:]],
                        outs=[agk_out[:]],
                        replica_groups=[[0, 1, 2, 3], [4, 5, 6, 7]],
                    )

                # ---- local V chunk projection + AllGather launch ----
                wv_sb = load_w("wv_sb", wv)
                vTl = inpool.tile([P, NIT, 512], dt.bfloat16, tag="inT")
                nc.sync.dma_start(
                    out=vTl, in_=vT.ap().rearrange("(t p) k -> p t k", p=P)
                )
                Vpl = vcpool.tile([P, 4, H * VW], dt.bfloat16, tag="Vpc")
                vones = Vpl.rearrange("p t (h x) -> p t h x", x=VW)[
                    :, :, :, DK : DK + 1
                ]
                nc.vector.memset(vones, 1.0)
                for rt in range(4):
                    for oc in range(2):
                        ps = psA.tile([P, 512], dt.float32, tag="ps1")
                        proj_group(
                            ps,
                            wv_sb,
                            vTl,
                            slice(rt * P, (rt + 1) * P),
                            slice(oc * 512, (oc + 1) * 512),
                            swap=True,
                        )
                        dst = Vpl[
                            :, rt, oc * 8 * VW : (oc * 8 + 8) * VW
                        ].rearrange("p (h x) -> p h x", x=VW)[:, :, 0:DK]
                        nc.vector.tensor_copy(
                            out=dst, in_=ps.rearrange("p (h x) -> p h x", x=DK)
                        )
                nc.sync.dma_start(
                    out=agv_in.ap().rearrange("(t p c) -> p t c", p=P, c=H * VW),
                    in_=Vpl,
                )
                if do_ag:
                    nc.gpsimd.collective_compute(
                        "AllGather",
                        mybir.AluOpType.bypass,
                        ins=[agv_in[:]],
                        outs=[agv_out[:]],
                        replica_groups=[[0, 1, 2, 3], [4, 5, 6, 7]],
                    )

              # ---- Q projection: Q.T[o, q] (runs while the AllGathers fly) ----
              if "q" in phases:
                  wq_sb = load_w("wq_sb", wq)
                  qT_sb = qin.tile([P, NIT, MQ], dt.bfloat16, tag="qTin")
                  nc.sync.dma_start(
                      out=qT_sb, in_=qT.ap().rearrange("(t p) q -> p t q", p=P)
                  )
                  for ot in range(NOT_):
                      ps = psA.tile([P, 512], dt.float32, tag="ps1")
                      proj_group(
                          ps, wq_sb, qT_sb, slice(ot * P, (ot + 1) * P), slice(None)
                      )
                      nc.vector.tensor_scalar_add(
                          out=QT_sb[:, ot, :], in0=ps, scalar1=bq_sb[:, ot : ot + 1]
                      )

              # ---- attention slices over the 4 chunks ----
              # Chunk r==partition_id's chunk is already in SBUF (KTl/Vpl) and
              # runs first; remote chunks are DMA'd from the AllGather outputs.
              # Since the NEFF is SPMD-identical per core, we read *all* chunks
              # from ag_out except that slice 0 uses the local SBUF tiles with
              # the mask column of the *local* chunk — we don't know L at
              # compile time, so maskb is passed pre-rotated per core: column
              # order [L, then the remaining chunks in ag_out order]. The av
              # accumulation order across chunks doesn't matter.
              if "attn" in phases and "kv" in phases:
                  def emit_norm(h):
                      recip = sm.tile([1, 512], dt.float32r, tag="recip")
                      with nc.allow_low_precision(
                          reason="fp32r keeps most of the mantissa"
                      ):
                          nc.vector.reciprocal(
                              out=recip, in_=av_acc[DK : DK + 1, h, :]
                          )
                      ps_bc = psA.tile([P, 512], dt.float32, tag="ps1")
                      nc.tensor.matmul(
                          ps_bc[0:DK, :],
                          lhsT=ones_sb,
                          rhs=recip,
                          start=True,
                          stop=True,
                      )
                      nc.vector.tensor_mul(
                          out=ctx_sb[(h % 2) * DK : (h % 2 + 1) * DK, h // 2, :],
                          in0=av_acc[0:DK, h, :],
                          in1=ps_bc[0:DK, :],
                      )

                  for si in range(NKC):
                      KTc = kcpool.tile(
                          [P, NOT_, 512], dt.bfloat16, tag="KTc", name=f"KTc{si}"
                      )
                      nc.sync.dma_start(
                          out=KTc,
                          in_=agk_out[
                              (si) * KHALF : (si + 1) * KHALF
                          ].rearrange("(t p k) -> p t k", p=P, k=512),
                      )
                      Vpc = vcpool.tile(
                          [P, 4, H * VW], dt.bfloat16, tag="Vpc", name=f"Vpc{si}"
                      )
                      nc.sync.dma_start(
                          out=Vpc,
                          in_=agv_out[
                              (si) * KHALFV : (si + 1) * KHALFV
                          ].rearrange("(t p c) -> p t c", p=P, c=H * VW),
                      )
                      def emit_av(j, av_tiles, p_tiles, rt):
                          for hh in range(2):
                              nc.tensor.matmul(
                                  av_tiles[hh][0:VW, :],
                                  lhsT=Vpc[
                                      :,
                                      rt,
                                      (2 * j + hh) * VW : (2 * j + hh + 1) * VW,
                                  ],
                                  rhs=p_tiles[rt][:, hh * 512 : (hh + 1) * 512],
                                  start=(rt == 0),
                                  stop=(rt == 3),
                                  skip_group_check=True,
                              )

                      def drain_pair(j, av_tiles):
                          for hh in range(2):
                              h = 2 * j + hh
                              if si == 0:
                                  nc.vector.tensor_copy(
                                      out=av_acc[0:VW, h, :],
                                      in_=av_tiles[hh][0:VW, :],
                                  )
                              else:
                                  nc.vector.tensor_add(
                                      out=av_acc[0:VW, h, :],
                                      in0=av_acc[0:VW, h, :],
                                      in1=av_tiles[hh][0:VW, :],
                                  )
                              if si == NKC - 1:
                                  emit_norm(h)

                      prev = None
                      for j in range(H // 2):
                          cur_av = [
                              psA.tile(
                                  [P, 512],
                                  dt.float32,
                                  tag="ps1",
                                  name=f"av_{si}_{j}_{m}",
                              )
                              for m in range(2)
                          ]
                          cur_p = []
                          for rt in range(4):
                              kt = si * 4 + rt
                              sc = psB.tile([P, 1024], dt.float32, tag="ps2")
                              nc.tensor.matmul(
                                  sc[:, 0:512],
                                  lhsT=KTc[0:DK, j, rt * P : (rt + 1) * P],
                                  rhs=QT_sb[0:DK, j, :],
                                  start=True,
                                  stop=True,
                                  tile_position=(0, 0),
                              )
                              nc.tensor.matmul(
                                  sc[:, 512:1024],
                                  lhsT=KTc[DK:P, j, rt * P : (rt + 1) * P],
                                  rhs=QT_sb[DK:P, j, :],
                                  start=True,
                                  stop=True,
                                  tile_position=(DK, 0),
                              )
                              p_kt = ppool.tile([P, 1024], dt.bfloat16, tag="pT")
                              nc.scalar.activation(
                                  out=p_kt,
                                  in_=sc,
                                  func=mybir.ActivationFunctionType.Exp,
                                  bias=mb_sb[:, kt : kt + 1],
                                  scale=1.0,
                              )
                              cur_p.append(p_kt)
                              if prev is not None:
                                  emit_av(prev[0], prev[1], prev[2], rt)
                          if prev is not None:
                              drain_pair(prev[0], prev[1])
                          prev = (j, cur_av, cur_p)
                      for rt in range(4):
                          emit_av(prev[0], prev[1], prev[2], rt)
                      drain_pair(prev[0], prev[1])


              # ---- output projection ----
              if "out" in phases:
                  wo_sb = load_w("wo_sb", wo)
                  for qt in range(NQT):
                      for oc in range(2):
                          ps = psA.tile([P, 512], dt.float32, tag="ps1")
                          for jt in range(NIT):
                              nc.tensor.matmul(
                                  ps,
                                  lhsT=ctx_sb[:, jt, qt * P : (qt + 1) * P],
                                  rhs=wo_sb[:, jt, oc * 512 : (oc + 1) * 512],
                                  start=(jt == 0),
                                  stop=(jt == NIT - 1),
                              )
                          o_sb = outp.tile([P, 512], dt.float32, tag="osb")
                          nc.vector.tensor_add(
                              out=o_sb,
                              in0=ps,
                              in1=bob_sb[:, oc * 512 : (oc + 1) * 512],
                          )
                          nc.sync.dma_start(
                              out=out[
                                  qt * P : (qt + 1) * P, oc * 512 : (oc + 1) * 512
                              ],
                              in_=o_sb,
                          )

            if _body_ctx is None:
                emit_body(do_ag=not sim_noag)
            else:
                emit_body(do_ag=True)
                _body_ctx.__enter__()
                emit_body(do_ag=False)
                _body_ctx.__exit__(None, None, None)

    nc.finalize()
    return nc


def _get_nc():
    if "nc" not in _CACHE:
        _CACHE["nc"] = _build()
    return _CACHE["nc"]


def _make_inputs(query, key, value, mask, wq, bq, wk, bk, wv, bv, wo, bo):
    query = np.asarray(query, dtype=np.float32)
    key = np.asarray(key, dtype=np.float32)
    value = np.asarray(value, dtype=np.float32)
    mask = np.asarray(mask)
    f32 = np.float32
    wqT = np.ascontiguousarray(np.asarray(wq, f32).T / 8.0).astype(BF16)
    wkT = np.ascontiguousarray(np.asarray(wk, f32).T).astype(BF16)
    wvT = np.ascontiguousarray(np.asarray(wv, f32).T).astype(BF16)
    woT = np.ascontiguousarray(np.asarray(wo, f32).T).astype(BF16)
    bq8 = np.ascontiguousarray((np.asarray(bq, f32) / 8.0).reshape(NOT_, P).T)
    bkr = np.ascontiguousarray(np.asarray(bk, f32).reshape(NOT_, P).T)
    bob = (np.asarray(bo, f32) + np.asarray(wo, f32) @ np.asarray(bv, f32))[None, :]
    bob = np.ascontiguousarray(bob)
    onesr = np.ones((1, DK), dtype=f32)

    in_maps = []
    for c in range(NCORES):
        b = c // 4
        L = c % 4
        q0 = L * MQ
        qTc = np.ascontiguousarray(query[b].T[:, q0 : q0 + MQ]).astype(BF16)
        kTc = np.ascontiguousarray(key[b].T[:, q0 : q0 + MQ]).astype(BF16)
        vTc = np.ascontiguousarray(value[b].T[:, q0 : q0 + MQ]).astype(BF16)
        mbias = np.where(mask[b, 0, 0] == 0, f32(-1e5), f32(0.0)).astype(f32)
        mbias = np.ascontiguousarray(mbias.reshape(NKT, P).T)
        in_maps.append(
            {
                "qT": qTc,
                "kT": kTc,
                "vT": vTc,
                "wq": wqT,
                "wk": wkT,
                "wv": wvT,
                "wo": woT,
                "bq": bq8,
                "bk": bkr,
                "maskb": mbias,
                "bob": bob,
                "onesr": onesr,
            }
        )
    return in_maps


def kernel(query, key, value, mask, wq, bq, wk, bk, wv, bv, wo, bo):
    import os
    from concourse.bass_utils import run_bass_kernel_spmd

    nc = _get_nc()
    in_maps = _make_inputs(
        query, key, value, mask, wq, bq, wk, bk, wv, bv, wo, bo
    )
    kw = {}
    if os.environ.get("KERNEL_TRACE"):
        kw = dict(trace=True, tmpdir=os.environ.get("KERNEL_TRACE_DIR") or None)
    res = run_bass_kernel_spmd(nc, in_maps, core_ids=list(range(NCORES)), **kw)
    _CACHE["last_res"] = res
    out = np.empty((B, S, D), dtype=np.float32)
    for c in range(NCORES):
        b = c // 4
        q0 = (c % 4) * MQ
        out[b, q0 : q0 + MQ, :] = res.results[c]["out"]
    return out



# revision 14
# speedup vs baseline: 1.4291x; 1.4291x over previous
"""Multi-head attention (B=2, S=2048, D=1024, H=16) on 8 trn2 NeuronCores.

Sharding: core c handles batch b=c//4 and query rows [512*(c%4), +512).
K/V projection of each core's own 512-row chunk is split into 4 head-pieces
(piece i = heads 4i..4i+3); each piece is AllGathered across the 4 cores of
the batch group as soon as it is projected, so the 8 small collectives
pipeline with the remaining projections and with attention piece 0..2.

Attention runs piece-by-piece (4 heads over ALL 2048 keys per piece): per
head-pair, scores (tile_position-packed K=64 matmuls) -> exp on ACT (mask
folded into the per-partition exp bias) -> attn@V accumulated IN PSUM across
all 16 key tiles (start/stop group), so no DVE adds and no av_acc SBUF
round-trip. The softmax denominator rides as a ones-column in V' (M=65) and
is placed at the low/high end per head parity so the context rows land on
the correct SBUF partitions without a partition shift.

Tail: 16 denominator rows are DMA-gathered into one [16,512] tile, a single
batched reciprocal + 8 block-broadcast matmuls + 8 DVE muls normalize the
context, then the output projection (K=128, full efficiency) writes out.

All device layouts are feature-major (inputs passed as x.T, weights as W.T);
1/sqrt(dk) is folded into wq/bq host-side; bv/bo folded into bo' = bo+wo@bv.
"""

import sys

for _p in ("/opt/trn_rl_repo", "/root/.axon_site/_ro/trn_rl_repo"):
    if _p not in sys.path:
        sys.path.insert(0, _p)

import numpy as np
import ml_dtypes

B, S, D, H, DK = 2, 2048, 1024, 16, 64
NCORES = 8
MQ = 512          # query rows per core
P = 128           # partitions
NOT_ = D // P     # 8 output-feature tiles
NIT = D // P      # 8 input-feature tiles
NKT = S // P      # 16 key tiles
VW = DK + 1       # 65: head dim + ones column
NPC = 4           # head-pieces (4 heads each)
KP = 2 * P * 512          # K piece elems (2 ot tiles, bf16)
VP = P * 4 * 4 * VW       # V piece elems (4 rt, 4 heads)

BF16 = ml_dtypes.bfloat16

_CACHE = {}


def _build():
    from concourse import bacc
    import concourse.mybir as mybir
    import concourse.tile as tile
    import concourse.bass as bass

    nc = bacc.Bacc("TRN2", target_bir_lowering=False, debug=False)
    dt = mybir.dt

    qT = nc.dram_tensor("qT", [D, MQ], dt.bfloat16, kind="ExternalInput")
    kT = nc.dram_tensor("kT", [D, 512], dt.bfloat16, kind="ExternalInput")
    vT = nc.dram_tensor("vT", [D, 512], dt.bfloat16, kind="ExternalInput")
    wq = nc.dram_tensor("wq", [D, D], dt.bfloat16, kind="ExternalInput")
    wk = nc.dram_tensor("wk", [D, D], dt.bfloat16, kind="ExternalInput")
    wv = nc.dram_tensor("wv", [D, D], dt.bfloat16, kind="ExternalInput")
    wo = nc.dram_tensor("wo", [D, D], dt.bfloat16, kind="ExternalInput")
    bq = nc.dram_tensor("bq", [P, NOT_], dt.float32, kind="ExternalInput")
    bk = nc.dram_tensor("bk", [P, NOT_], dt.float32, kind="ExternalInput")
    maskb = nc.dram_tensor("maskb", [P, NKT], dt.float32, kind="ExternalInput")
    bob = nc.dram_tensor("bob", [1, D], dt.float32, kind="ExternalInput")
    blkones = nc.dram_tensor("blkones", [16, 8 * P], dt.float32r, kind="ExternalInput")
    out = nc.dram_tensor("out", [MQ, D], dt.float32, kind="ExternalOutput")
    import os as _os
    _dbg = bool(_os.environ.get("KERNEL_DEBUG"))
    if _dbg:
        dbg_den = nc.dram_tensor("dbg_den", [16, MQ], dt.float32, kind="ExternalOutput")
        dbg_denw = nc.dram_tensor("dbg_denw", [1, H * MQ], dt.float32, kind="ExternalOutput")
        dbg_ctx = nc.dram_tensor("dbg_ctx", [P, NOT_ * MQ], dt.bfloat16, kind="ExternalOutput")
        dbg_kt = nc.dram_tensor("dbg_kt", [P, 2 * 4 * 512], dt.bfloat16, kind="ExternalOutput")
        dbg_vp = nc.dram_tensor("dbg_vp", [P, 4 * 4 * 4 * VW], dt.bfloat16, kind="ExternalOutput")

    den_dram = nc.dram_tensor("den_dram", [H * MQ], dt.float32)
    agk_in = [nc.dram_tensor(f"agk_in{i}", [KP], dt.bfloat16) for i in range(NPC)]
    agk_out = [nc.dram_tensor(f"agk_out{i}", [4 * KP], dt.bfloat16) for i in range(NPC)]
    agv_in = [nc.dram_tensor(f"agv_in{i}", [VP], dt.bfloat16) for i in range(NPC)]
    agv_out = [nc.dram_tensor(f"agv_out{i}", [4 * VP], dt.bfloat16) for i in range(NPC)]

    RG = [[0, 1, 2, 3], [4, 5, 6, 7]]

    with tile.TileContext(nc) as tc:
        with (
            tc.tile_pool(name="w", bufs=3) as wpool,
            tc.tile_pool(name="stat", bufs=1) as stat,
            tc.tile_pool(name="inT", bufs=1) as inpool,
            tc.tile_pool(name="big", bufs=1) as big,
            tc.tile_pool(name="kc", bufs=2) as kcpool,
            tc.tile_pool(name="vc", bufs=2) as vcpool,
            tc.tile_pool(name="pT", bufs=8) as ppool,
            tc.tile_pool(name="outp", bufs=2) as outp,
            tc.tile_pool(name="sc", bufs=2, space="PSUM") as scpool,
            tc.tile_pool(name="av", bufs=4, space="PSUM") as avpool,
        ):
            # ---- persistent tiles ----
            QT_sb = big.tile([P, NOT_, MQ], dt.bfloat16, tag="QT")
            KTl = big.tile([P, NOT_, 512], dt.bfloat16, tag="KTl")
            Vpl = big.tile([P, 4, H * VW], dt.bfloat16, tag="Vpl")
            ctx_sb = big.tile([P, NOT_, MQ], dt.bfloat16, tag="ctx")
            den16 = big.tile([16, MQ], dt.float32, tag="den16")
            denw = big.tile([1, H * MQ], dt.float32, tag="denw")
            recip16 = big.tile([16, MQ], dt.float32r, tag="recip16")
            bq_sb = stat.tile([P, NOT_], dt.float32, tag="bq")
            bk_sb = stat.tile([P, NOT_], dt.float32, tag="bk")
            mb_sb = stat.tile([P, NKT], dt.float32, tag="mb")
            bob_sb = stat.tile([P, D], dt.float32, tag="bob")
            blk_sb = stat.tile([16, 8 * P], dt.float32r, tag="blk")

            # inputs / weights: spread across the two HW DGE queues
            wk_sb = wpool.tile([P, NIT, D], dt.bfloat16, tag="w", name="wk_sb")
            nc.sync.dma_start(out=wk_sb, in_=wk.ap().rearrange("(t p) o -> p t o", p=P))
            kTl = inpool.tile([P, NIT, 512], dt.bfloat16, tag="kin")
            nc.scalar.dma_start(out=kTl, in_=kT.ap().rearrange("(t p) k -> p t k", p=P))
            wv_sb = wpool.tile([P, NIT, D], dt.bfloat16, tag="w", name="wv_sb")
            nc.sync.dma_start(out=wv_sb, in_=wv.ap().rearrange("(t p) o -> p t o", p=P))
            vTl = inpool.tile([P, NIT, 512], dt.bfloat16, tag="vin")
            nc.scalar.dma_start(out=vTl, in_=vT.ap().rearrange("(t p) k -> p t k", p=P))
            wq_sb = wpool.tile([P, NIT, D], dt.bfloat16, tag="w", name="wq_sb")
            nc.sync.dma_start(out=wq_sb, in_=wq.ap().rearrange("(t p) o -> p t o", p=P))
            qTl = inpool.tile([P, NIT, MQ], dt.bfloat16, tag="qin")
            nc.scalar.dma_start(out=qTl, in_=qT.ap().rearrange("(t p) q -> p t q", p=P))

            nc.sync.dma_start(out=bq_sb, in_=bq[:, :])
            nc.sync.dma_start(out=bk_sb, in_=bk[:, :])
            nc.sync.dma_start(out=mb_sb, in_=maskb[:, :])
            bob_bcast = bass.AP(tensor=bob.ap().tensor, offset=0, ap=[[0, P], [1, D]])
            nc.sync.dma_start(out=bob_sb, in_=bob_bcast)
            nc.sync.dma_start(out=blk_sb, in_=blkones[:, :])

            # ones columns of V' (position DK for even heads, 0 for odd)
            vview = Vpl.rearrange("p r (h x) -> p r h x", x=VW)
            nc.vector.memset(vview[:, :, :, DK : DK + 1], 1.0)

            def mm_group(ps, w_sb, x_sb, m_slice, n_slice, swap=False):
                for it in range(NIT):
                    lhsT = x_sb[:, it, m_slice] if swap else w_sb[:, it, m_slice]
                    rhs = w_sb[:, it, n_slice] if swap else x_sb[:, it, n_slice]
                    nc.tensor.matmul(
                        ps, lhsT=lhsT, rhs=rhs,
                        start=(it == 0), stop=(it == NIT - 1),
                    )

            # ---- K/V projections piece-by-piece, AGs launched ASAP ----
            for i in range(NPC):
                # K piece: ot = 2i, 2i+1
                for ot in (2 * i, 2 * i + 1):
                    ps = avpool.tile([P, 512], dt.float32, tag="av", name=f"psk{ot}")
                    mm_group(ps, wk_sb, kTl, slice(ot * P, (ot + 1) * P), slice(None))
                    nc.vector.tensor_scalar_add(
                        out=KTl[:, ot, :], in0=ps, scalar1=bk_sb[:, ot : ot + 1]
                    )
                nc.sync.dma_start(
                    out=agk_in[i].ap().rearrange("(t p k) -> p t k", p=P, k=512),
                    in_=KTl[:, 2 * i : 2 * i + 2, :],
                )
                nc.gpsimd.collective_compute(
                    "AllGather", mybir.AluOpType.bypass,
                    ins=[agk_in[i][:]],
                    outs=[agk_out[i][:]],
                    replica_groups=RG,
                )
                # V piece: heads 4i..4i+3 (feature cols [4i*64, +256))
                for rt in range(4):
                    psf = avpool.tile([P, 512], dt.float32, tag="av", name=f"psv{i}{rt}")
                    ps = psf[:, 0:256]
                    mm_group(
                        ps, wv_sb, vTl,
                        slice(rt * P, (rt + 1) * P),
                        slice(i * 256, (i + 1) * 256),
                        swap=True,
                    )
                    pv = ps.rearrange("p (l f) -> p l f", f=DK)
                    for l in range(4):
                        h = 4 * i + l
                        nc.vector.tensor_copy(
                            out=vview[:, rt, h, 0:DK], in_=pv[:, l, :]
                        )
                nc.sync.dma_start(
                    out=agv_in[i].ap().rearrange("(p r v) -> p r v", p=P, v=4 * VW),
                    in_=vview[:, :, 4 * i : 4 * i + 4, :].rearrange(
                        "p r h x -> p r (h x)"
                    ),
                )
                nc.gpsimd.collective_compute(
                    "AllGather", mybir.AluOpType.bypass,
                    ins=[agv_in[i][:]],
                    outs=[agv_out[i][:]],
                    replica_groups=RG,
                )

            # ---- Q projection (all heads) ----
            for ot in range(NOT_):
                ps = avpool.tile([P, 512], dt.float32, tag="av", name=f"psq{ot}")
                mm_group(ps, wq_sb, qTl, slice(ot * P, (ot + 1) * P), slice(None))
                nc.vector.tensor_scalar_add(
                    out=QT_sb[:, ot, :], in0=ps, scalar1=bq_sb[:, ot : ot + 1]
                )

            # wo load early so the output projection is never DMA-gated
            wo_sb = wpool.tile([P, NIT, D], dt.bfloat16, tag="w", name="wo_sb")
            nc.scalar.dma_start(out=wo_sb, in_=wo.ap().rearrange("(t p) o -> p t o", p=P))

            # ---- attention, piece by piece ----
            for i in range(NPC):
                KTp = kcpool.tile([P, 2, 4, 512], dt.bfloat16, tag="KTp", name=f"KTp{i}")
                kin = agk_out[i].ap().rearrange(
                    "(c t p k) -> t p c k", t=2, p=P, k=512
                )
                for t in range(2):
                    nc.scalar.dma_start(out=KTp[:, t, :, :], in_=kin[t])
                Vpp = vcpool.tile([P, 4, 4, 4 * VW], dt.bfloat16, tag="Vpp", name=f"Vpp{i}")
                for cc in range(4):
                    nc.sync.dma_start(
                        out=Vpp[:, cc, :, :],
                        in_=agv_out[i][cc * VP : (cc + 1) * VP].rearrange(
                            "(p r v) -> p r v", p=P, v=4 * VW
                        ),
                    )
                for jp in range(2):
                    ot = 2 * i + jp
                    av = [
                        avpool.tile([P, 512], dt.float32, tag="av", name=f"av{i}{jp}{m}")
                        for m in range(2)
                    ]
                    step = 0
                    for c in range(4):
                        for rt in range(4):
                            kt = c * 4 + rt
                            sc = scpool.tile([P, 1024], dt.float32, tag="sc")
                            nc.tensor.matmul(
                                sc[:, 0:512],
                                lhsT=KTp[0:DK, jp, c, rt * P : (rt + 1) * P],
                                rhs=QT_sb[0:DK, ot, :],
                                start=True, stop=True, tile_position=(0, 0),
                            )
                            nc.tensor.matmul(
                                sc[:, 512:1024],
                                lhsT=KTp[DK:P, jp, c, rt * P : (rt + 1) * P],
                                rhs=QT_sb[DK:P, ot, :],
                                start=True, stop=True, tile_position=(DK, 0),
                            )
                            p_t = ppool.tile([P, 1024], dt.bfloat16, tag="pT")
                            nc.scalar.activation(
                                out=p_t, in_=sc,
                                func=mybir.ActivationFunctionType.Exp,
                                bias=mb_sb[:, kt : kt + 1], scale=1.0,
                            )
                            for hh in range(2):
                                l = 2 * jp + hh
                                nc.tensor.matmul(
                                    av[hh][0:VW, :],
                                    lhsT=Vpp[:, c, rt, l * VW : (l + 1) * VW],
                                    rhs=p_t[:, hh * 512 : (hh + 1) * 512],
                                    start=(step == 0), stop=(step == 15),
                                    skip_group_check=True,
                                )
                            step += 1
                    # drain pair: ctx rows + denominator gather
                    blk = 2 * i + jp
                    for hh in range(2):
                        h = 4 * i + 2 * jp + hh
                        nc.vector.tensor_copy(
                            out=denw[0:1, h * MQ : (h + 1) * MQ],
                            in_=av[hh][DK : DK + 1, :],
                        )
                        nc.vector.tensor_copy(
                            out=ctx_sb[hh * DK : (hh + 1) * DK, blk, :],
                            in_=av[hh][0:DK, :],
                        )

            if _dbg:
                nc.sync.dma_start(out=dbg_kt[:, :], in_=KTp.rearrange("p a b c -> p (a b c)"))
                nc.sync.dma_start(out=dbg_vp[:, :], in_=Vpp.rearrange("p a b c -> p (a b c)"))

            # ---- normalize: one batched reciprocal + block broadcasts ----
            nc.sync.dma_start(
                out=den_dram.ap().rearrange("(o x) -> o x", o=1), in_=denw
            )
            nc.sync.dma_start(
                out=den16, in_=den_dram.ap().rearrange("(h q) -> h q", q=MQ)
            )
            if _dbg:
                nc.sync.dma_start(out=dbg_den[:, :], in_=den16)
                nc.sync.dma_start(out=dbg_denw[:, :], in_=denw)
                nc.sync.dma_start(out=dbg_ctx[:, :], in_=ctx_sb.rearrange("p a b -> p (a b)"))
            with nc.allow_low_precision(reason="fp32r keeps most of the mantissa"):
                nc.vector.reciprocal(out=recip16, in_=den16)
            for j in range(8):
                bc = avpool.tile([P, 512], dt.float32, tag="av", name=f"bc{j}")
                nc.tensor.matmul(
                    bc, lhsT=blk_sb[:, j * P : (j + 1) * P], rhs=recip16,
                    start=True, stop=True,
                )
                nc.vector.tensor_mul(
                    out=ctx_sb[:, j, :], in0=ctx_sb[:, j, :], in1=bc
                )

            # ---- output projection ----
            for qt in range(4):
                for oc in range(2):
                    ps = avpool.tile([P, 512], dt.float32, tag="av", name=f"o{qt}{oc}")
                    for jt in range(NIT):
                        nc.tensor.matmul(
                            ps,
                            lhsT=ctx_sb[:, jt, qt * P : (qt + 1) * P],
                            rhs=wo_sb[:, jt, oc * 512 : (oc + 1) * 512],
                            start=(jt == 0), stop=(jt == NIT - 1),
                        )
                    o_sb = outp.tile([P, 512], dt.float32, tag="osb")
                    nc.vector.tensor_add(
                        out=o_sb, in0=ps, in1=bob_sb[:, oc * 512 : (oc + 1) * 512]
                    )
                    nc.sync.dma_start(
                        out=out[qt * P : (qt + 1) * P, oc * 512 : (oc + 1) * 512],
                        in_=o_sb,
                    )

    nc.finalize()
    return nc


def _get_nc():
    if "nc" not in _CACHE:
        _CACHE["nc"] = _build()
    return _CACHE["nc"]


def _make_inputs(query, key, value, mask, wq, bq, wk, bk, wv, bv, wo, bo):
    query = np.asarray(query, dtype=np.float32)
    key = np.asarray(key, dtype=np.float32)
    value = np.asarray(value, dtype=np.float32)
    mask = np.asarray(mask)
    f32 = np.float32
    wqT = np.ascontiguousarray(np.asarray(wq, f32).T / 8.0).astype(BF16)
    wkT = np.ascontiguousarray(np.asarray(wk, f32).T).astype(BF16)
    wvT = np.ascontiguousarray(np.asarray(wv, f32).T).astype(BF16)
    woT = np.ascontiguousarray(np.asarray(wo, f32).T).astype(BF16)
    bq8 = np.ascontiguousarray((np.asarray(bq, f32) / 8.0).reshape(NOT_, P).T)
    bkr = np.ascontiguousarray(np.asarray(bk, f32).reshape(NOT_, P).T)
    bob = (np.asarray(bo, f32) + np.asarray(wo, f32) @ np.asarray(bv, f32))[None, :]
    bob = np.ascontiguousarray(bob)
    blk = np.zeros((16, 8 * P), dtype=f32)
    for j in range(8):
        blk[2 * j, j * P : j * P + DK] = 1.0
        blk[2 * j + 1, j * P + DK : (j + 1) * P] = 1.0

    in_maps = []
    for c in range(NCORES):
        b = c // 4
        L = c % 4
        q0 = L * MQ
        qTc = np.ascontiguousarray(query[b].T[:, q0 : q0 + MQ]).astype(BF16)
        kTc = np.ascontiguousarray(key[b].T[:, q0 : q0 + MQ]).astype(BF16)
        vTc = np.ascontiguousarray(value[b].T[:, q0 : q0 + MQ]).astype(BF16)
        mbias = np.where(mask[b, 0, 0] == 0, f32(-1e5), f32(0.0)).astype(f32)
        mbias = np.ascontiguousarray(mbias.reshape(NKT, P).T)
        in_maps.append(
            {
                "qT": qTc,
                "kT": kTc,
                "vT": vTc,
                "wq": wqT,
                "wk": wkT,
                "wv": wvT,
                "wo": woT,
                "bq": bq8,
                "bk": bkr,
                "maskb": mbias,
                "bob": bob,
                "blkones": blk,
            }
        )
    return in_maps


def kernel(query, key, value, mask, wq, bq, wk, bk, wv, bv, wo, bo):
    import os
    from concourse.bass_utils import run_bass_kernel_spmd

    nc = _get_nc()
    in_maps = _make_inputs(
        query, key, value, mask, wq, bq, wk, bk, wv, bv, wo, bo
    )
    kw = {}
    if os.environ.get("KERNEL_TRACE"):
        kw = dict(trace=True, tmpdir=os.environ.get("KERNEL_TRACE_DIR") or None)
    res = run_bass_kernel_spmd(nc, in_maps, core_ids=list(range(NCORES)), **kw)
    _CACHE["last_res"] = res
    out = np.empty((B, S, D), dtype=np.float32)
    for c in range(NCORES):
        b = c // 4
        q0 = (c % 4) * MQ
        out[b, q0 : q0 + MQ, :] = res.results[c]["out"]
    return out
